# revision 3
# baseline (speedup 1.0000x reference)
"""Trainium2 Bass kernel for nn_DTRN — time-parallel speculative version.

Sharding: each of the 8 cores computes a 256-step time slice of the full
sequence for the FULL batch (B=16), using speculative warmup: every scan
chain starts W steps (per dependency level) before/after the slice from a
zero state; the LSTM dynamics contract (forget gate + 0.9-bounded discount
gating), so the state converges to the true orbit well within W steps.
x is zero-padded outside [0,T); with zero biases the padded region keeps the
state exactly zero, so the edge cores reproduce the reference exactly.

Chain ranges in local coords (origin = t0 - 3W, slice = [3W, 3W+SL)):
  d_f  [0, SL+4W)      d_b  [2W, SL+6W)   (b = processed descending)
  l0_f [W, SL+4W)      l0_b [2W, SL+5W)
  l1_f [2W, SL+3W)     l1_b [3W, SL+4W)

Phase A runs d and l0 (both dirs) in lockstep rounds over S-step blocks
(same t-window per round for all four chains; l0 joins W/S rounds in).
Phase B runs l1 both dirs. Same per-step micro-kernel as the baseline:
gates in PSUM chunk-layout [128, 8*BL], weight-stationary bf16 recurrent
matmuls, SIG2 trick (tanh via doubled-g sigmoid), GPSIMD offload for the
backward chains' elementwise tail.
"""

import sys

sys.path.insert(0, "/opt/trn_rl_repo")

from contextlib import ExitStack

import numpy as np
import ml_dtypes

import concourse.bass as bass
import concourse.tile as tile
from concourse import bacc, mybir
from concourse import bass_utils

F32 = mybir.dt.float32
BF16 = mybir.dt.bfloat16
FP8 = mybir.dt.float8e4
AF = mybir.ActivationFunctionType
DF = 0.9

# recurrent weights in fp8 e4m3, scaled by FP8_SCALE (recovered by the
# sigmoid's scale argument); input-projection path scaled to match
FP8_WHH = False
FP8_SCALE = 64.0

B, T_FULL, I, H = 16, 2048, 256, 256
NCORES = 8
BL = B                  # full batch per core
SL = T_FULL // NCORES   # 256-step slice per core
W = 48                  # speculative warmup window
S = 16                  # steps per block
G4 = 4 * H
NCH = G4 // 128         # 8 gate chunks
LB = SL + 6 * W         # local x extent
E_D = SL + 6 * W        # d_b top (exclusive)
E_L0 = SL + 5 * W
E_L1 = SL + 4 * W
NR_A = (SL + 4 * W) // S
LAG = W // S
NR_B = (SL + W) // S
NXB = 3                 # x-stream ring slots (of S steps each)
L0LEN = SL + 2 * W      # l0 hist buffer (wrapped)
L1LEN = SL + W

# gate-permutation: (i,f,g,o) -> (i,f,o,g); g rows pre-doubled (SIG2)
_PERM = np.r_[0:2 * H, 3 * H:4 * H, 2 * H:3 * H]


def _ds(e, n):
    return bass.ds(e, n)


class _Builder:
    def __init__(self):
        self.nc = bacc.Bacc("TRN2", target_bir_lowering=False, debug=False)

    # ---------------- DRAM I/O ----------------
    def declare_io(self):
        nc = self.nc
        self.d_xT = nc.dram_tensor("xT", [2 * 128, LB, BL], BF16, kind="ExternalInput").ap()
        # per-core validity mask over local time (1 inside [0,T), 0 in padding)
        self.d_hmask = nc.dram_tensor("hmask", [1, LB], BF16, kind="ExternalInput").ap()
        self.d_w = {}
        for ph, nk in (("d", 2), ("l0", 2), ("l1", 4)):
            for dr in ("f", "b"):
                self.d_w[f"{ph}wih_{dr}"] = nc.dram_tensor(
                    f"{ph}wih_{dr}", [nk * 128, G4], BF16, kind="ExternalInput").ap()
                self.d_w[f"{ph}whh_{dr}"] = nc.dram_tensor(
                    f"{ph}whh_{dr}", [2 * 128, G4], FP8 if FP8_WHH else BF16,
                    kind="ExternalInput").ap()
                self.d_w[f"{ph}bias_{dr}"] = nc.dram_tensor(
                    f"{ph}bias_{dr}", [128, NCH], F32, kind="ExternalInput").ap()
        for dr in ("f", "b"):
            self.d_w[f"cwT_{dr}"] = nc.dram_tensor(
                f"cwT_{dr}", [2 * 128, 1], BF16, kind="ExternalInput").ap()
            self.d_w[f"cb_{dr}"] = nc.dram_tensor(
                f"cb_{dr}", [1, 1], F32, kind="ExternalInput").ap()
        self.d_ident = nc.dram_tensor("ident", [128, 128], BF16, kind="ExternalInput").ap()
        self.d_ones = nc.dram_tensor("ones", [1, 128], BF16, kind="ExternalInput").ap()
        self.d_out = nc.dram_tensor("out", [BL, SL, 2 * H], F32, kind="ExternalOutput").ap()

    # ---------------- index maps ----------------
    @staticmethod
    def _hidx(ph, dr, t):
        if ph == "d":
            return t % (2 * S)
        if ph == "l0":
            return (t - 2 * W) % L0LEN
        return (t - 2 * W) if dr == "f" else (t - 3 * W)

    @staticmethod
    def _cfidx(dr, t):
        return (t - W) if dr == "f" else (t - 2 * W)

    # ---------------- build ----------------
    def build(self, ctx: ExitStack, tc: tile.TileContext):
        nc = self.nc
        self.tc = tc

        persist = ctx.enter_context(tc.tile_pool(name="persist", bufs=1))
        wpool = ctx.enter_context(tc.tile_pool(name="weights", bufs=1))
        self.wpool = wpool
        self.psum_g = ctx.enter_context(tc.tile_pool(name="psum_g", bufs=2, space="PSUM"))
        self.psum_x = ctx.enter_context(tc.tile_pool(name="psum_x", bufs=2, space="PSUM"))
        self.psum_c = ctx.enter_context(tc.tile_pool(name="psum_c", bufs=2, space="PSUM"))
        self.xp_pool = ctx.enter_context(tc.tile_pool(name="xp_stage", bufs=2))
        self.cfb_pool = ctx.enter_context(tc.tile_pool(name="cfb", bufs=2))
        self.gp = ctx.enter_context(tc.tile_pool(name="gates", bufs=2))
        self.outp = ctx.enter_context(tc.tile_pool(name="outstage", bufs=4))

        # --- constants ---
        self.ident = persist.tile([128, 128], BF16, tag="ident", name="ident")
        nc.sync.dma_start(self.ident[:], self.d_ident)
        self.ones = persist.tile([1, 128], BF16, tag="ones", name="ones")
        nc.sync.dma_start(self.ones[:], self.d_ones)
        self.zrow = persist.tile([128, 2, BL], BF16, tag="zrow", name="zrow")
        nc.vector.memset(self.zrow[:], 0.0)

        # broadcast the validity mask across partitions: [128, LB]
        hm = persist.tile([1, LB], BF16, tag="hm", name="hm")
        nc.sync.dma_start(hm[:], self.d_hmask)
        self.maskb = persist.tile([128, LB], BF16, tag="maskb", name="maskb")
        mchunk = LB // ((LB + 511) // 512)
        assert LB % mchunk == 0
        for h0 in range(0, LB, mchunk):
            pm = self.psum_c.tile([128, mchunk], F32, tag="pc", name="pm")
            nc.tensor.matmul(pm[:], lhsT=self.ones[:], rhs=hm[:, h0:h0 + mchunk],
                             start=True, stop=True)
            nc.vector.tensor_copy(self.maskb[:, h0:h0 + mchunk], pm[:])

        # --- x stream rings (shared by d and l0; one per direction) ---
        self.xs = {}
        for dr in ("f", "b"):
            self.xs[dr] = persist.tile([128, 2, NXB * S, BL], BF16,
                                       tag=f"xs_{dr}", name=f"xs_{dr}")

        # --- histories ---
        self.hist = {}
        for dr in ("f", "b"):
            self.hist[("d", dr)] = persist.tile(
                [128, 2, 2 * S, BL], BF16, tag=f"hist_d_{dr}", name=f"hist_d_{dr}")
            self.hist[("l0", dr)] = persist.tile(
                [128, 2, L0LEN, BL], BF16, tag=f"hist_l0_{dr}", name=f"hist_l0_{dr}")
            self.hist[("l1", dr)] = persist.tile(
                [128, 2, L1LEN, BL], BF16, tag=f"hist_l1_{dr}", name=f"hist_l1_{dr}")

        # --- coefficients: f origin W, b origin 2W ---
        self.cf = {dr: persist.tile([1, SL + 3 * W, BL], BF16, tag=f"cf_{dr}",
                                    name=f"cf_{dr}")
                   for dr in ("f", "b")}

        # --- c state per chain ---
        self.c_sb = {}
        for ph in ("d", "l0", "l1"):
            for dr in ("f", "b"):
                self.c_sb[(ph, dr)] = persist.tile(
                    [128, 2, BL], F32, tag=f"c_{ph}_{dr}", name=f"c_{ph}_{dr}")

        # --- coefficient weights ---
        self.cwT = {}
        self.cb = {}
        for dr in ("f", "b"):
            t = persist.tile([128, 2, 1], BF16, tag=f"cwT_{dr}", name=f"cwT_{dr}")
            for k in range(2):
                nc.sync.dma_start(t[:, k], self.d_w[f"cwT_{dr}"][k * 128:(k + 1) * 128])
            self.cwT[dr] = t
            tb = persist.tile([1, 1], F32, tag=f"cb_{dr}", name=f"cb_{dr}")
            nc.sync.dma_start(tb[:], self.d_w[f"cb_{dr}"])
            self.cb[dr] = tb

        # --- phase A: d + l0 ---
        wd = self.load_phase_weights("d", 2)
        wl0 = self.load_phase_weights("l0", 2)
        for widx in range(min(2, NR_A)):
            self.x_dma(widx)
        for r in range(NR_A):
            if r + 2 < NR_A:
                self.x_dma(r + 2)
            fw0 = r * S
            bw0 = E_D - (r + 1) * S
            self.block("d", 2, *wd, {"f": fw0, "b": bw0},
                       {"f": r == 0, "b": r == 0}, xslot=r % NXB)
            if r >= LAG:
                self.coeff_round(fw0, bw0)
                self.block("l0", 2, *wl0, {"f": fw0, "b": bw0},
                           {"f": r == LAG, "b": r == LAG}, xslot=r % NXB)

        # --- phase B: l1 (reuses d's weight slots) ---
        wl1 = self.load_phase_weights("l1", 4)
        for q in range(NR_B):
            fw0 = 2 * W + q * S
            bw0 = E_L1 - (q + 1) * S
            self.block("l1", 4, *wl1, {"f": fw0, "b": bw0},
                       {"f": q == 0, "b": q == 0})
            if q == 11:
                # these output chunks are already complete; overlap their
                # transposes with the remaining l1 rounds
                self.write_out([("f", 0), ("b", 1)])

        self.write_out([("f", 1), ("b", 0)])

    # ---------------- weights ----------------
    def load_phase_weights(self, ph, nk):
        nc = self.nc
        pg = "l0" if ph == "l0" else "A"
        wih, whh, bias = {}, {}, {}
        for dr in ("f", "b"):
            w1 = self.wpool.tile([128, 4 if pg == "A" else nk, G4], BF16,
                                 tag=f"wih_{pg}_{dr}", name=f"{ph}wih_{dr}")
            for k in range(nk):
                nc.sync.dma_start(w1[:, k], self.d_w[f"{ph}wih_{dr}"][k * 128:(k + 1) * 128])
            wih[dr] = w1
            w2 = self.wpool.tile([128, 2, G4], FP8 if FP8_WHH else BF16,
                                 tag=f"whh_{pg}_{dr}", name=f"{ph}whh_{dr}")
            for k in range(2):
                nc.sync.dma_start(w2[:, k], self.d_w[f"{ph}whh_{dr}"][k * 128:(k + 1) * 128])
            whh[dr] = w2
            bt = self.wpool.tile([128, NCH], F32, tag=f"bias_{pg}_{dr}", name=f"{ph}bias_{dr}")
            nc.sync.dma_start(bt[:], self.d_w[f"{ph}bias_{dr}"])
            bias[dr] = bt
            nc.vector.memset(self.c_sb[(ph, dr)][:], 0.0)
        return wih, whh, bias

    # ---------------- x streaming ----------------
    def x_dma(self, widx):
        nc = self.nc
        slot = widx % NXB
        for dr in ("f", "b"):
            w0 = widx * S if dr == "f" else E_D - (widx + 1) * S
            for k in range(2):
                nc.sync.dma_start(
                    self.xs[dr][:, k, slot * S:(slot + 1) * S, :],
                    self.d_xT[k * 128:(k + 1) * 128, w0:w0 + S, :])

    # ---------------- xp source ----------------
    def xp_rhs(self, ph, dr, kc, w0, xslot):
        if ph in ("d", "l0"):
            return self.xs[dr][:, kc, xslot * S:(xslot + 1) * S, :]
        src = self.hist[("l0", "f")] if kc < 2 else self.hist[("l0", "b")]
        m0 = (w0 - 2 * W) % L0LEN
        return src[:, kc % 2, m0:m0 + S, :]

    # ---------------- coefficient round ----------------
    def coeff_round(self, fw0, bw0):
        nc = self.nc
        for dr, w0 in (("f", fw0), ("b", bw0)):
            hd = self.hist[("d", dr)]
            m0 = w0 % (2 * S)
            pc = self.psum_c.tile([1, S * BL], F32, tag="pc", name="coefb")
            for kc in range(2):
                nc.tensor.matmul(
                    pc[:],
                    lhsT=self.cwT[dr][:, kc],
                    rhs=hd[:, kc, m0:m0 + S, :],
                    start=(kc == 0), stop=(kc == 1),
                )
            c0 = self._cfidx(dr, w0)
            cfl = self.cf[dr][:, c0:c0 + S, :]
            nc.scalar.activation(cfl, pc[:], AF.Sigmoid, bias=self.cb[dr][:])
            nc.vector.tensor_scalar_mul(cfl, cfl, DF)

    # ---------------- one S-step block for both dirs of a phase ----------------
    def block(self, ph, nk, wih, whh, bias, wins, firsts, xslot=None):
        nc = self.nc
        gated = ph != "d"
        xp = {}
        cfb = {}
        for dr in ("f", "b"):
            w0 = wins[dr]
            stage = self.xp_pool.tile(
                [128, S, NCH, BL], BF16,
                tag=f"xp_{'l0' if ph == 'l0' else 'A'}_{dr}", name=f"xp_{dr}")
            for m in range(NCH):
                px = self.psum_x.tile([128, S * BL], F32, tag="px", name="px")
                for kc in range(nk):
                    nc.tensor.matmul(
                        px[:],
                        lhsT=wih[dr][:, kc, m * 128:(m + 1) * 128],
                        rhs=self.xp_rhs(ph, dr, kc, w0, xslot),
                        start=(kc == 0),
                        stop=(kc == nk - 1),
                    )
                nc.scalar.activation(
                    stage[:, :, m, :], px[:], AF.Identity,
                    bias=bias[dr][:, m:m + 1],
                    scale=FP8_SCALE if FP8_WHH else 1.0)
            xp[dr] = stage
            if gated:
                c0 = self._cfidx(dr, w0)
                pc = self.psum_c.tile([128, S * BL], F32, tag="pc", name="pc")
                nc.tensor.matmul(
                    pc[:],
                    lhsT=self.ones[:],
                    rhs=self.cf[dr][:, c0:c0 + S, :],
                    start=True, stop=True,
                )
                cfv = self.cfb_pool.tile([128, S, BL], BF16, tag=f"cfb_{dr}",
                                         name=f"cfb_{dr}")
                nc.vector.tensor_copy(cfv[:], pc[:])
                cfb[dr] = cfv

        pg = "l0" if ph == "l0" else "A"
        # ---- the S scan steps, f/b interleaved, paired xp injection ----
        gpair = {}
        for s in range(S):
            for dr in ("f", "b"):
                if dr == "f":
                    sx = s
                    t = wins["f"] + s
                    tprev = t - 1
                else:
                    sx = S - 1 - s
                    t = wins["b"] + sx
                    tprev = t + 1
                hist = self.hist[(ph, dr)]
                at_start = firsts[dr] and s == 0
                hi = self._hidx(ph, dr, t)
                hn = hist[:, :, _ds(hi, 1), :]
                if at_start:
                    hprev = self.zrow[:]
                else:
                    hp = self._hidx(ph, dr, tprev)
                    hprev = hist[:, :, _ds(hp, 1), :]

                # one injection matmul seeds the gate psum for two steps
                if s % 2 == 0:
                    gp2 = self.psum_g.tile([128, 2, NCH * BL], F32,
                                           tag=f"g_{dr}", name="g")
                    lo = sx if dr == "f" else sx - 1
                    nc.tensor.matmul(
                        gp2[:], lhsT=self.ident[:],
                        rhs=xp[dr][:, lo:lo + 2],
                        start=True, stop=False, skip_group_check=True,
                    )
                    gpair[dr] = gp2
                g = gpair[dr][:, sx % 2]
                for kc in range(2):
                    hk = self.zrow[:, kc] if at_start else hprev[:, kc]
                    for m in range(NCH):
                        nc.tensor.matmul(
                            g[:, m * BL:(m + 1) * BL],
                            lhsT=whh[dr][:, kc, m * 128:(m + 1) * 128],
                            rhs=hk,
                            start=False,
                            stop=(kc == 1 and m == NCH - 1 and s % 2 == 1),
                            skip_group_check=True,
                        )

                ve = nc.gpsimd if dr == "b" else nc.vector
                c = self.c_sb[(ph, dr)][:]
                cf2 = None
                if gated:
                    cfs = cfb[dr][:, sx, :]
                    cf2 = bass.AP(
                        tensor=cfs.tensor,
                        offset=cfs.offset,
                        ap=[list(cfs.ap[0]), [0, 2], list(cfs.ap[1])],
                    )
                MUL = mybir.AluOpType.mult

                # SIG2: one sigmoid covers i,f,o and 2g (g-rows pre-doubled)
                sig = self.gp.tile([128, 8 * BL], F32, tag=f"sig_{pg}_{dr}",
                                   name=f"sig_{dr}")
                nc.scalar.activation(sig[:], g[:], AF.Sigmoid,
                                     scale=(1.0 / FP8_SCALE) if FP8_WHH else 1.0)
                si, sf, so = sig[:, :2 * BL], sig[:, 2 * BL:4 * BL], sig[:, 4 * BL:6 * BL]
                s2g = sig[:, 6 * BL:]
                a1 = self.gp.tile([128, 2 * BL], F32, tag=f"t1_{pg}_{dr}", name=f"a1_{dr}")
                nc.vector.scalar_tensor_tensor(a1[:], s2g, 2.0, si, MUL, MUL)  # 2*si*s2g
                t1 = self.gp.tile([128, 2 * BL], F32, tag=f"tg_{pg}_{dr}", name=f"t1_{dr}")
                ve.tensor_sub(t1[:], a1[:], si)                  # si*tanh(g)
                a3 = self.gp.tile([128, 2 * BL], F32, tag=f"t2_{pg}_{dr}", name=f"a3_{dr}")
                ve.tensor_mul(a3[:], sf, c)                      # f*c
                ve.tensor_add(c, t1[:], a3[:])                   # c_n
                s2c = self.gp.tile([128, 2 * BL], F32, tag=f"tc_{pg}_{dr}", name=f"s2c_{dr}")
                nc.scalar.activation(s2c[:], c, AF.Sigmoid, scale=2.0)
                b1 = self.gp.tile([128, 2 * BL], F32, tag=f"ho_{pg}_{dr}", name=f"b1_{dr}")
                nc.vector.scalar_tensor_tensor(b1[:], s2c[:], 2.0, so, MUL, MUL)  # 2*so*s2c
                tail = nc.gpsimd
                if not gated:
                    tail.tensor_sub(hn, b1[:], so)               # o*tanh(c)
                else:
                    b2 = self.gp.tile([128, 2 * BL], F32, tag=f"b2_{pg}_{dr}", name=f"b2_{dr}")
                    tail.tensor_sub(b2[:], b1[:], so)
                    u = self.gp.tile([128, 2 * BL], F32, tag=f"u_{pg}_{dr}", name=f"u_{dr}")
                    tail.tensor_mul(u[:], cf2, hprev)            # cf*h_prev
                    tail.tensor_add(hn, b2[:], u[:])

        # l0 feeds l1: zero its history in padding regions (edge cores) so
        # l1's warmup sees the reference's zero-input extension
        if ph == "l0":
            for dr in ("f", "b"):
                w0 = wins[dr]
                m0 = (w0 - 2 * W) % L0LEN
                hb = self.hist[("l0", dr)][:, :, m0:m0 + S, :]
                ms = self.maskb[:, w0:w0 + S]
                mask_ap = bass.AP(
                    tensor=ms.tensor,
                    offset=ms.offset,
                    ap=[list(ms.ap[0]), [0, 2], list(ms.ap[1]), [0, BL]],
                )
                nc.vector.tensor_mul(hb, hb, mask_ap)

    # ---------------- output ----------------
    def write_out(self, parts):
        nc = self.nc
        TB = 128
        for dr, tb in parts:
            dr_i = 0 if dr == "f" else 1
            hist = self.hist[("l1", dr)]
            off0 = W if dr == "f" else 0  # local out range start within l1 hist
            for k in range(2):
                for b in range(BL):
                    if True:
                        ps = self.psum_x.tile([TB, 128], BF16, tag="px", name="tr")
                        nc.tensor.transpose(
                            ps[:], hist[:, k, off0 + tb * TB:off0 + (tb + 1) * TB, b],
                            self.ident[:])
                        st = self.outp.tile([TB, 128], F32, tag="st", name="st")
                        nc.vector.tensor_copy(st[:], ps[:])
                        nc.sync.dma_start(
                            self.d_out[b, tb * TB:(tb + 1) * TB,
                                       dr_i * 256 + k * 128:dr_i * 256 + (k + 1) * 128],
                            st[:])


# ================= host-side program cache =================
_PROG_CACHE = {}


def _build_program():
    key = ("main",)
    if key in _PROG_CACHE:
        return _PROG_CACHE[key]
    b = _Builder()
    b.declare_io()
    with tile.TileContext(b.nc) as tc:
        with ExitStack() as ctx:
            b.build(ctx, tc)
    b.nc.compile()
    _PROG_CACHE[key] = b
    return b


def _build_null_program():
    """Same I/O signature, trivial compute — measures dispatch/RTT floor."""
    key = ("null",)
    if key in _PROG_CACHE:
        return _PROG_CACHE[key]
    b = _Builder()
    b.declare_io()
    nc = b.nc
    with tile.TileContext(nc) as tc:
        with ExitStack() as ctx:
            pool = ctx.enter_context(tc.tile_pool(name="p", bufs=2))
            t = pool.tile([128, 512], F32, name="t")
            nc.sync.dma_start(t[:], b.d_out[0, 0:128, :])
            nc.vector.tensor_scalar_mul(t[:], t[:], 1.0)
            nc.sync.dma_start(b.d_out[0, 0:128, :], t[:])
    nc.compile()
    _PROG_CACHE[key] = b
    return b


# ================= host-side data prep =================
def _prep_weights(kw):
    out = {}
    for ph, pre, nk in (("d", "d", 2), ("l0", "l0", 2), ("l1", "l1", 4)):
        for dr in ("f", "b"):
            Wih = np.asarray(kw[f"{pre}Wih_{dr}"], np.float32)[_PERM]     # [4H, in]
            Whh = np.asarray(kw[f"{pre}Whh_{dr}"], np.float32)[_PERM]     # [4H, H]
            bias = (np.asarray(kw[f"{pre}bih_{dr}"], np.float32)
                    + np.asarray(kw[f"{pre}bhh_{dr}"], np.float32))[_PERM]
            # SIG2: double g-gate rows so one sigmoid yields sigmoid(2g)
            Wih = Wih.copy(); Whh = Whh.copy(); bias = bias.copy()
            Wih[6 * 128:] *= 2.0
            Whh[6 * 128:] *= 2.0
            bias[6 * 128:] *= 2.0
            out[f"{ph}wih_{dr}"] = np.ascontiguousarray(Wih.T).astype(ml_dtypes.bfloat16)
            if FP8_WHH:
                out[f"{ph}whh_{dr}"] = np.ascontiguousarray(
                    Whh.T * FP8_SCALE).astype(ml_dtypes.float8_e4m3)
                bias = bias * FP8_SCALE
            else:
                out[f"{ph}whh_{dr}"] = np.ascontiguousarray(Whh.T).astype(ml_dtypes.bfloat16)
            out[f"{ph}bias_{dr}"] = np.ascontiguousarray(
                bias.reshape(NCH, 128).T).astype(np.float32)
    for dr in ("f", "b"):
        out[f"cwT_{dr}"] = np.ascontiguousarray(
            np.asarray(kw[f"cw_{dr}"], np.float32).reshape(1, H).T).astype(ml_dtypes.bfloat16)
        out[f"cb_{dr}"] = np.asarray(kw[f"cb_{dr}"], np.float32).reshape(1, 1)
    out["ident"] = np.eye(128, dtype=ml_dtypes.bfloat16)
    out["ones"] = np.ones((1, 128), dtype=ml_dtypes.bfloat16)
    return out


def _in_maps(inputs_np):
    x = np.asarray(inputs_np["inputs"], np.float32)           # [B, T, I]
    wmaps = _prep_weights(inputs_np)
    x_pad = np.zeros((B, T_FULL + 6 * W, I), np.float32)
    x_pad[:, 3 * W:3 * W + T_FULL] = x
    maps = []
    for c in range(NCORES):
        xs = x_pad[:, c * SL:c * SL + LB]                     # [B, LB, I]
        xT = np.ascontiguousarray(xs.transpose(2, 1, 0)).astype(ml_dtypes.bfloat16)
        m = dict(wmaps)
        m["xT"] = xT
        real_t = c * SL - 3 * W + np.arange(LB)
        m["hmask"] = ((real_t >= 0) & (real_t < T_FULL)).astype(
            ml_dtypes.bfloat16).reshape(1, LB)
        maps.append(m)
    return maps


def _run(inputs_np, trace=False, trace_kwargs=None):
    in_maps = _in_maps(inputs_np)
    prog = _build_program()
    res = bass_utils.run_bass_kernel_spmd(
        prog.nc, in_maps, list(range(NCORES)), trace=trace,
        **(trace_kwargs or {}))
    outs = np.concatenate([res.results[c]["out"] for c in range(NCORES)], axis=1)
    return outs, res


def kernel(**inputs):
    out, _ = _run(inputs)
    return out.astype(np.float32)
